# revision 31
# baseline (speedup 1.0000x reference)
"""Trainium2 Bass kernel for MeshGNN message passing (8 NeuronCores, SPMD).

Math reformulation (exact): since softmax weights sum to 1 and the output MLP is
linear, fold W_concat/W_out into per-node quantities:
    M1 = W_out @ W_concat[:, :128]   [3,128]
    M2 = W_out @ W_concat[:, 128:]   [3,3]
    c0 = b_concat @ W_out.T + b_out  [3]
    kx[j] = x[j] @ W_k.T + b_k                  (64,)   -> table
    w[j]  = x[j] @ M1.T + p[j] @ M2.T           (3,)    -> table
    q[n]  = (x[n] @ W_q.T + b_q) / scale        (64,)
    scores[n,k] = q[n] . kx[nbr]
    e = exp(scores * (nbr != 0))                         (scores bounded ~3)
    out[n] = base[n] + sum_k e_k * w[nbr] / sum_k e_k,
    base = p + c0 - p @ M2.T                             (host-computed)

Per-node table rows of 128 fp16 (=256B): [kx(64)|w(3)|pad61], laid out
partition-major (row = p*n_tiles + t) so the table write is one contiguous
stream. Rows are fetched with dma_gather in PAIRS (512B, idx = row//2 fits
int16; negative int16 idxs are broken on HW so exact row indexing is
impossible). The pair-half select is folded into the softmax:
e2m[s] = pmask[s]*exp(s2a[s]*cmask[s]) is exact for the selected half, 0 for
the wrong half, and exp(0)=1 for padding edges (matching the reference's
softmax(scores*mask) semantics).

Phase 1 computes the own-shard table with one matmul per 128-node tile (3
tiles per PSUM bank) and fused DVE copy+bias; an AllGather shares the fp16
table. Desc-gen on the Q7 SWDGE (~2.7ns/idx, engine-serial) is the floor for
phase 2; dummy warmup gathers at t=0 pull the Q7 dma_gather IRAM overlay into
each queue's CPU pair so the first real chunks run at full rate.
"""

import sys

import numpy as np

sys.path.insert(0, "/opt/trn_rl_repo")

import concourse.bass as bass
import concourse.mybir as mybir
import concourse.tile as tile
from concourse import bacc
from concourse.bass import ds, ts
from concourse.bass_utils import run_bass_kernel_spmd

N_CORES = 8
H = 128
K = 15
DT = mybir.dt
F16 = DT.float16
F32 = DT.float32
I16 = DT.int16

ROW = 128                 # fp16 elems per table row (256B)
PAIR = 2 * ROW            # gather element: two rows (512B)
QC = 64                   # q/k dim
W1C = 131                 # matmul cols: kx(64)|w(3)|q(64)
S = 2 * K                 # pair-expanded slots per node


def build_program(n_total, shard, n_tiles, chunk_tiles=2):
    P = 128
    nc = bacc.Bacc(None, debug=False, num_swdge_queues=4)

    ax = nc.declare_dram_parameter("ax", [P, shard], F16, isOutput=False)  # x.T
    idx16 = nc.declare_dram_parameter("idx16", [P, n_tiles * K * 8], I16,
                                      isOutput=False)
    cmask = nc.declare_dram_parameter("cmask", [P, n_tiles * K * 2], F32,
                                      isOutput=False)
    pmask = nc.declare_dram_parameter("pmask", [P, n_tiles * K * 2], F16,
                                      isOutput=False)
    w1 = nc.declare_dram_parameter("w1", [P, W1C], F16, isOutput=False)
    brow = nc.declare_dram_parameter("brow", [P, W1C], F16, isOutput=False)
    pxp = nc.declare_dram_parameter("pxp", [P, n_tiles * 3], F16,
                                    isOutput=False)
    basep = nc.declare_dram_parameter("basep", [P, n_tiles * 3], F32,
                                      isOutput=False)
    out = nc.declare_dram_parameter("out", [P, n_tiles * 3], F32, isOutput=True)

    with tile.TileContext(nc) as tc:
        with (
            tc.tile_pool(name="persist", bufs=1) as pp,
            tc.tile_pool(name="dram", bufs=1, space="DRAM") as dp,
            tc.tile_pool(name="psum", bufs=4, space="PSUM") as psp,
            tc.tile_pool(name="kxgp", bufs=6) as kxgp,
            tc.tile_pool(name="work", bufs=2) as wp,
        ):
            # ---- persistent SBUF ----
            xT = pp.tile([P, shard], F16)
            idx_sb = pp.tile([P, n_tiles * K * 8], I16)
            cm_sb = pp.tile([P, n_tiles * K * 2], F32)
            pm_sb = pp.tile([P, n_tiles * K * 2], F16)
            w1_sb = pp.tile([P, W1C], F16)
            br_sb = pp.tile([P, W1C], F16)
            px_sb = pp.tile([P, n_tiles * 3], F16)
            base_sb = pp.tile([P, n_tiles * 3], F32)
            q_sb = pp.tile([P, n_tiles * QC], F16)
            out_sb = pp.tile([P, n_tiles * 3], F32)
            tblall_sb = pp.tile([P, n_tiles * ROW], F16)

            # table rows laid out partition-major: DRAM row = p*n_tiles + t,
            # so the table write is a contiguous per-partition stream.
            table_pad = dp.tile([shard, ROW], F16, space="DRAM")
            table_full = dp.tile([N_CORES * shard, ROW], F16, space="DRAM",
                                 addr_space="Shared")

            # Full-size dummy gathers: keep the Q7/SWDGE + DMA engines under
            # sustained load during phase 1 so the first real chunks don't pay
            # the slow-start (clock-ramp-like) penalty observed after ~100us
            # of gather inactivity.
            wu_n = chunk_tiles * K * P
            wu_idx = pp.tile([P, wu_n // 16], I16)
            wu_dst = pp.tile([P, (wu_n // P) * ROW], F16)
            nc.vector.memset(wu_idx[:], 0)
            ax_rows = ax[:, :].rearrange("p (a e) -> (p a) e", e=ROW)
            for q in range(8):
                nc.gpsimd.dma_gather(
                    wu_dst[:].rearrange("p (s e) -> p s e", e=ROW),
                    ax_rows,
                    wu_idx[:, :],
                    wu_n, wu_n, ROW,
                    single_packet=False,
                    queue_num=q % 4,
                )

            nc.sync.dma_start(out=xT[:], in_=ax[:, :])
            nc.sync.dma_start(out=idx_sb[:], in_=idx16[:, :])
            nc.sync.dma_start(out=cm_sb[:], in_=cmask[:, :])
            nc.sync.dma_start(out=pm_sb[:], in_=pmask[:, :])
            nc.sync.dma_start(out=w1_sb[:], in_=w1[:, :])
            nc.sync.dma_start(out=br_sb[:], in_=brow[:, :])
            nc.sync.dma_start(out=px_sb[:], in_=pxp[:, :])
            nc.sync.dma_start(out=base_sb[:], in_=basep[:, :])

            nc.vector.memset(tblall_sb[:], 0)

            # ---- phase 1: x@W1 (3 tiles per PSUM bank), DVE copy+bias ----
            tbl3 = tblall_sb[:].rearrange("p (t e) -> p t e", e=ROW)
            q3 = q_sb[:].rearrange("p (t e) -> p t e", e=QC)
            NT = n_tiles
            GROUP = 3
            g0 = 0
            while g0 < n_tiles:
                gn = min(GROUP, n_tiles - g0)
                ps = psp.tile([P, GROUP * W1C], F32, space="PSUM", tag="ps")
                for i in range(gn):
                    nc.tensor.matmul(out=ps[:, ds(i * W1C, W1C)],
                                     lhsT=xT[:, ts(g0 + i, P)], rhs=w1_sb[:],
                                     start=True, stop=True)
                psv = ps[:].rearrange("p (g c) -> p g c", c=W1C)
                nc.vector.tensor_tensor(
                    out=tbl3[:, g0:g0 + gn, 0:67], in0=psv[:, 0:gn, 0:67],
                    in1=br_sb[:, 0:67].unsqueeze(1).broadcast_to([P, gn, 67]),
                    op=mybir.AluOpType.add)
                nc.vector.tensor_tensor(
                    out=q3[:, g0:g0 + gn, :], in0=psv[:, 0:gn, 67:W1C],
                    in1=br_sb[:, 67:W1C].unsqueeze(1).broadcast_to([P, gn, QC]),
                    op=mybir.AluOpType.add)
                g0 += gn
            nc.vector.tensor_tensor(
                out=tbl3[:, :, 64:67], in0=tbl3[:, :, 64:67],
                in1=px_sb[:].rearrange("p (t e) -> p t e", e=3),
                op=mybir.AluOpType.add)

            # one contiguous DMA for the whole table slice
            nc.sync.dma_start(
                out=table_pad[:].rearrange("(p t) e -> p (t e)", p=P),
                in_=tblall_sb[:])

            # ---- all-gather the fp16 table (full shard incl. padding) ----
            nc.gpsimd.collective_compute(
                "AllGather",
                mybir.AluOpType.bypass,
                replica_groups=[list(range(N_CORES))],
                ins=[table_pad[:, :].opt()],
                outs=[table_full[:].opt()],
            )

            table_pairs = table_full[:].rearrange("(a two) e -> a (two e)",
                                                  two=2)
            # odd-sized chunk first: it lands in the slow post-collective
            # window and keeps the tail uniform
            chunks = []
            t0 = 0
            if n_tiles % chunk_tiles:
                chunks.append((0, n_tiles % chunk_tiles))
                t0 = n_tiles % chunk_tiles
            while t0 < n_tiles:
                chunks.append((t0, chunk_tiles))
                t0 += chunk_tiles

            # ---- phase 2: pair-gather + attention ----
            for ci, (t0, nt) in enumerate(chunks):
                nidx = nt * K * P
                kxg = kxgp.tile([P, nt * K * PAIR], F16, tag="kxg")
                nc.gpsimd.dma_gather(
                    kxg[:].rearrange("p (s e) -> p s e", e=PAIR),
                    table_pairs,
                    idx_sb[:, ds(t0 * K * 8, nt * K * 8)],
                    nidx, nidx, PAIR,
                    single_packet=False,
                    queue_num=ci % 4,
                )
                kx4 = kxg[:].rearrange("p (t s e) -> p t s e", s=S, e=ROW)
                qc_ap = (q_sb[:, ds(t0 * QC, nt * QC)]
                         .rearrange("p (t e) -> p t e", e=QC)
                         .unsqueeze(2).broadcast_to([P, nt, S, QC]))
                prod = wp.tile([P, nt * S * QC], F16, tag="prod")
                pr5 = prod[:].rearrange("p (t s e) -> p t s e", s=S, e=QC)
                nc.vector.tensor_tensor(out=pr5, in0=kx4[:, :, :, 0:QC],
                                        in1=qc_ap, op=mybir.AluOpType.mult)
                s2a = wp.tile([P, nt * S], F16, tag="s2a")
                with nc.allow_low_precision("scores ~|3| fit fp16"):
                    nc.vector.tensor_reduce(out=s2a[:], in_=pr5,
                                            axis=mybir.AxisListType.X,
                                            op=mybir.AluOpType.add)
                sm2a = wp.tile([P, nt * S], F16, tag="sm2a")
                nc.vector.tensor_tensor(out=sm2a[:], in0=s2a[:],
                                        in1=cm_sb[:, ds(t0 * S, nt * S)],
                                        op=mybir.AluOpType.mult)
                ea2 = wp.tile([P, nt * S], F16, tag="ea2")
                nc.scalar.activation(out=ea2[:], in_=sm2a[:],
                                     func=mybir.ActivationFunctionType.Exp)
                e2m = wp.tile([P, nt * S], F16, tag="e2m")
                nc.vector.tensor_tensor(out=e2m[:], in0=ea2[:],
                                        in1=pm_sb[:, ds(t0 * S, nt * S)],
                                        op=mybir.AluOpType.mult)
                sea = wp.tile([P, nt], F32, tag="sea")
                nc.vector.tensor_reduce(
                    out=sea[:], in_=e2m[:].rearrange("p (t s) -> p t s", s=S),
                    axis=mybir.AxisListType.X, op=mybir.AluOpType.add)
                ra = wp.tile([P, nt], F32, tag="ra")
                nc.vector.reciprocal(out=ra[:], in_=sea[:])
                # e-major layout [p, t, 3, s]: long inner runs for both ops
                wpr = wp.tile([P, nt * 3 * S], F32, tag="wpr")
                nc.vector.tensor_tensor(
                    out=wpr[:].rearrange("p (t e s) -> p t e s", s=S, e=3),
                    in0=kx4[:, :, :, QC:QC + 3]
                        .rearrange("p t s e -> p t e s"),
                    in1=e2m[:].rearrange("p (t s) -> p t s", s=S)
                        .unsqueeze(2).broadcast_to([P, nt, 3, S]),
                    op=mybir.AluOpType.mult)
                wsum = wp.tile([P, nt * 3], F32, tag="wsum")
                nc.vector.tensor_reduce(
                    out=wsum[:],
                    in_=wpr[:].rearrange("p (t e s) -> p t e s", s=S, e=3),
                    axis=mybir.AxisListType.X, op=mybir.AluOpType.add)
                disp = wp.tile([P, nt * 3], F32, tag="disp")
                nc.vector.tensor_tensor(
                    out=disp[:].rearrange("p (t e) -> p t e", e=3),
                    in0=wsum[:].rearrange("p (t e) -> p t e", e=3),
                    in1=ra[:].unsqueeze(2).broadcast_to([P, nt, 3]),
                    op=mybir.AluOpType.mult)
                nc.vector.tensor_tensor(
                    out=out_sb[:, ds(t0 * 3, nt * 3)], in0=disp[:],
                    in1=base_sb[:, ds(t0 * 3, nt * 3)],
                    op=mybir.AluOpType.add)

            nc.sync.dma_start(out=out[:, :], in_=out_sb[:])

    nc.finalize()
    return nc


def prep_inputs(sampled_points, sampled_x, edge_index_filtered,
                W_concat, b_concat, W_out, b_out, W_q, b_q, W_k, b_k,
                n_total, shard, n_tiles):
    """Host-side layout prep + weight folding. Returns in_maps for 8 cores."""
    P = 128
    scale = np.sqrt(np.float32(H // 2), dtype=np.float32) + 1e-6

    Wc = W_concat.astype(np.float64)
    Wo = W_out.astype(np.float64)
    M1 = Wo @ Wc[:, :H]                                    # [3,128]
    M2 = Wo @ Wc[:, H:]                                    # [3,3]
    c0 = b_concat.astype(np.float64) @ Wo.T + b_out.astype(np.float64)

    w1 = np.zeros((P, W1C), np.float64)
    w1[:, 0:64] = W_k.astype(np.float64).T
    w1[:, 64:67] = M1.T
    w1[:, 67:W1C] = W_q.astype(np.float64).T / scale
    w1 = w1.astype(np.float16)

    brow = np.zeros((1, W1C), np.float64)
    brow[0, 0:64] = b_k.astype(np.float64)
    brow[0, 67:W1C] = b_q.astype(np.float64) / scale
    brow_rep = np.repeat(brow.astype(np.float16), P, 0)

    dst = np.asarray(edge_index_filtered[1]).reshape(n_total, K)
    valid = n_total // N_CORES

    # node j -> partition-major table row: owning core r, local (t, p) slot,
    # row = r*shard + p*n_tiles + t
    def row_of(j):
        r_core = j // valid
        loc = j % valid
        return r_core * shard + (loc % P) * n_tiles + loc // P

    in_maps = []
    for r in range(N_CORES):
        rows = slice(r * valid, (r + 1) * valid)
        x_r = np.zeros((shard, H), np.float16)
        x_r[:valid] = sampled_x[rows].astype(np.float16)
        nb_r = np.zeros((shard, K), np.int64)
        nb_r[:valid] = dst[rows]
        pt_r = np.zeros((shard, 3), np.float64)
        pt_r[:valid] = sampled_points[rows].astype(np.float64)

        px = (pt_r @ M2.T).astype(np.float16)
        base = (pt_r + c0[None, :] - pt_r @ M2.T).astype(np.float32)

        def swz(a, width):
            return (a.reshape(n_tiles, P, width).transpose(1, 0, 2)
                    .reshape(P, n_tiles * width).copy())

        # gather indices: position (slot = t*K+k, p) -> idx = row//2, stored
        # int16 wrapped-16: [16, pos//16] replicated to all 8 partition groups
        nbs = nb_r.reshape(n_tiles, P, K)
        npos = n_tiles * K * P
        pos = np.arange(npos)
        slot, p = pos // P, pos % P
        t_, k_ = slot // K, slot % K
        stream = row_of(nbs[t_, p, k_])
        idxw = (stream // 2).astype(np.int16).reshape(-1, 16).T  # [16, npos/16]
        idx_rep = np.tile(idxw, (8, 1))                          # [128, npos/16]

        par = (stream % 2).astype(np.float32)                    # h=1 half
        nz = (nbs[t_, p, k_] != 0).astype(np.float32)
        # masks laid out [p, (t k h)]
        pmask = np.zeros((P, n_tiles * K * 2), np.float32)
        pmask[p, (t_ * K + k_) * 2 + 0] = 1.0 - par
        pmask[p, (t_ * K + k_) * 2 + 1] = par
        cmask = pmask.copy()
        cmask[p, (t_ * K + k_) * 2 + 0] *= nz
        cmask[p, (t_ * K + k_) * 2 + 1] *= nz

        in_maps.append({
            "ax": np.ascontiguousarray(x_r.T),
            "idx16": np.ascontiguousarray(idx_rep),
            "cmask": cmask.astype(np.float32),
            "pmask": pmask.astype(np.float16),
            "w1": w1,
            "brow": brow_rep,
            "pxp": swz(px, 3),
            "basep": swz(base, 3),
        })
    return in_maps


def assemble_output(results, n_total, n_tiles):
    P = 128
    valid = n_total // N_CORES
    outs = []
    for r in range(N_CORES):
        o = results[r]["out"]
        o = (o.reshape(P, n_tiles, 3).transpose(1, 0, 2)
             .reshape(n_tiles * P, 3)[:valid])
        outs.append(o)
    return np.concatenate(outs, axis=0).astype(np.float32)


_CACHED = {}


def _get_program(n_total, shard, n_tiles):
    key = (n_total, shard, n_tiles)
    if key not in _CACHED:
        _CACHED[key] = build_program(n_total, shard, n_tiles)
    return _CACHED[key]


def kernel(sampled_points, sampled_x, edge_index_filtered,
           W_concat, b_concat, W_out, b_out, W_q, b_q, W_k, b_k):
    n_total = 60000
    n_tiles = 59
    shard = n_tiles * 128
    nc = _get_program(n_total, shard, n_tiles)
    in_maps = prep_inputs(
        np.asarray(sampled_points), np.asarray(sampled_x),
        np.asarray(edge_index_filtered),
        np.asarray(W_concat), np.asarray(b_concat),
        np.asarray(W_out), np.asarray(b_out),
        np.asarray(W_q), np.asarray(b_q),
        np.asarray(W_k), np.asarray(b_k),
        n_total, shard, n_tiles)
    res = run_bass_kernel_spmd(nc, in_maps, list(range(N_CORES)))
    return assemble_output(res.results, n_total, n_tiles)


# revision 32
# speedup vs baseline: 2.4822x; 2.4822x over previous
"""Trainium2 Bass kernel for MeshGNN message passing (8 NeuronCores, SPMD).

Math reformulation (exact): since softmax weights sum to 1 and the output MLP is
linear, fold W_concat/W_out into per-node quantities:
    M1 = W_out @ W_concat[:, :128]   [3,128]
    M2 = W_out @ W_concat[:, 128:]   [3,3]
    c0 = b_concat @ W_out.T + b_out  [3]
    kx[j] = x[j] @ W_k.T + b_k                  (64,)   -> table
    w[j]  = x[j] @ M1.T + p[j] @ M2.T           (3,)    -> table
    q[n]  = (x[n] @ W_q.T + b_q) / scale        (64,)
    scores[n,k] = q[n] . kx[nbr]
    e = exp(scores * (nbr != 0))                         (scores bounded ~3)
    out[n] = base[n] + sum_k e_k * w[nbr] / sum_k e_k,
    base = p + c0 - p @ M2.T                             (host-computed)

Per-node table rows of 128 fp16 (=256B): [kx(64)|w(3)|pad61], laid out
partition-major (row = p*n_tiles + t) so the table write is one contiguous
stream. Rows are fetched with dma_gather in PAIRS (512B, idx = row//2 fits
int16; negative int16 idxs are broken on HW so exact row indexing is
impossible). The pair-half select is folded into the softmax:
e2m[s] = pmask[s]*exp(s2a[s]*cmask[s]) is exact for the selected half, 0 for
the wrong half, and exp(0)=1 for padding edges (matching the reference's
softmax(scores*mask) semantics).

Phase 1 computes the own-shard table with one matmul per 128-node tile (3
tiles per PSUM bank) and fused DVE copy+bias; an AllGather shares the fp16
table. Desc-gen on the Q7 SWDGE (~2.7ns/idx, engine-serial) is the floor for
phase 2; dummy warmup gathers at t=0 pull the Q7 dma_gather IRAM overlay into
each queue's CPU pair so the first real chunks run at full rate.
"""

import sys

import numpy as np

sys.path.insert(0, "/opt/trn_rl_repo")

import concourse.bass as bass
import concourse.mybir as mybir
import concourse.tile as tile
from concourse import bacc
from concourse.bass import ds, ts
from concourse.bass_utils import run_bass_kernel_spmd

N_CORES = 8
H = 128
K = 15
DT = mybir.dt
F16 = DT.float16
F32 = DT.float32
I16 = DT.int16

ROW = 128                 # fp16 elems per table row (256B)
PAIR = 2 * ROW            # gather element: two rows (512B)
QC = 64                   # q/k dim
W1C = 131                 # matmul cols: kx(64)|w(3)|q(64)
S = 2 * K                 # pair-expanded slots per node


def build_program(n_total, shard, n_tiles, chunk_tiles=2):
    P = 128
    nc = bacc.Bacc(None, debug=False, num_swdge_queues=4)

    ax = nc.declare_dram_parameter("ax", [P, shard], F16, isOutput=False)  # x.T
    idx16 = nc.declare_dram_parameter("idx16", [P, n_tiles * K * 8], I16,
                                      isOutput=False)
    cmask = nc.declare_dram_parameter("cmask", [P, n_tiles * K * 2], F32,
                                      isOutput=False)
    pmask = nc.declare_dram_parameter("pmask", [P, n_tiles * K * 2], F16,
                                      isOutput=False)
    w1 = nc.declare_dram_parameter("w1", [P, W1C], F16, isOutput=False)
    brow = nc.declare_dram_parameter("brow", [P, W1C], F16, isOutput=False)
    pxp = nc.declare_dram_parameter("pxp", [P, n_tiles * 3], F16,
                                    isOutput=False)
    basep = nc.declare_dram_parameter("basep", [P, n_tiles * 3], F32,
                                      isOutput=False)
    out = nc.declare_dram_parameter("out", [P, n_tiles * 3], F32, isOutput=True)

    with tile.TileContext(nc) as tc:
        with (
            tc.tile_pool(name="persist", bufs=1) as pp,
            tc.tile_pool(name="dram", bufs=1, space="DRAM") as dp,
            tc.tile_pool(name="psum", bufs=4, space="PSUM") as psp,
            tc.tile_pool(name="kxgp", bufs=6) as kxgp,
            tc.tile_pool(name="work", bufs=2) as wp,
        ):
            # ---- persistent SBUF ----
            xT = pp.tile([P, shard], F16)
            idx_sb = pp.tile([P, n_tiles * K * 8], I16)
            cm_sb = pp.tile([P, n_tiles * K * 2], F32)
            pm_sb = pp.tile([P, n_tiles * K * 2], F16)
            w1_sb = pp.tile([P, W1C], F16)
            br_sb = pp.tile([P, W1C], F16)
            px_sb = pp.tile([P, n_tiles * 3], F16)
            base_sb = pp.tile([P, n_tiles * 3], F32)
            q_sb = pp.tile([P, n_tiles * QC], F16)
            out_sb = pp.tile([P, n_tiles * 3], F32)
            tblall_sb = pp.tile([P, n_tiles * ROW], F16)

            # table rows laid out partition-major: DRAM row = p*n_tiles + t,
            # so the table write is a contiguous per-partition stream.
            table_pad = dp.tile([shard, ROW], F16, space="DRAM")
            table_full = dp.tile([N_CORES * shard, ROW], F16, space="DRAM",
                                 addr_space="Shared")

            nc.sync.dma_start(out=xT[:], in_=ax[:, :])
            nc.sync.dma_start(out=idx_sb[:], in_=idx16[:, :])
            nc.sync.dma_start(out=cm_sb[:], in_=cmask[:, :])
            nc.sync.dma_start(out=pm_sb[:], in_=pmask[:, :])
            nc.sync.dma_start(out=w1_sb[:], in_=w1[:, :])
            nc.sync.dma_start(out=br_sb[:], in_=brow[:, :])
            nc.sync.dma_start(out=px_sb[:], in_=pxp[:, :])
            nc.sync.dma_start(out=base_sb[:], in_=basep[:, :])

            nc.vector.memset(tblall_sb[:], 0)

            # ---- phase 1: x@W1 (3 tiles per PSUM bank), DVE copy+bias ----
            tbl3 = tblall_sb[:].rearrange("p (t e) -> p t e", e=ROW)
            q3 = q_sb[:].rearrange("p (t e) -> p t e", e=QC)
            NT = n_tiles
            GROUP = 3
            g0 = 0
            while g0 < n_tiles:
                gn = min(GROUP, n_tiles - g0)
                ps = psp.tile([P, GROUP * W1C], F32, space="PSUM", tag="ps")
                for i in range(gn):
                    nc.tensor.matmul(out=ps[:, ds(i * W1C, W1C)],
                                     lhsT=xT[:, ts(g0 + i, P)], rhs=w1_sb[:],
                                     start=True, stop=True)
                psv = ps[:].rearrange("p (g c) -> p g c", c=W1C)
                nc.vector.tensor_tensor(
                    out=tbl3[:, g0:g0 + gn, 0:67], in0=psv[:, 0:gn, 0:67],
                    in1=br_sb[:, 0:67].unsqueeze(1).broadcast_to([P, gn, 67]),
                    op=mybir.AluOpType.add)
                nc.vector.tensor_tensor(
                    out=q3[:, g0:g0 + gn, :], in0=psv[:, 0:gn, 67:W1C],
                    in1=br_sb[:, 67:W1C].unsqueeze(1).broadcast_to([P, gn, QC]),
                    op=mybir.AluOpType.add)
                g0 += gn
            nc.vector.tensor_tensor(
                out=tbl3[:, :, 64:67], in0=tbl3[:, :, 64:67],
                in1=px_sb[:].rearrange("p (t e) -> p t e", e=3),
                op=mybir.AluOpType.add)

            # one contiguous DMA for the whole table slice
            nc.sync.dma_start(
                out=table_pad[:].rearrange("(p t) e -> p (t e)", p=P),
                in_=tblall_sb[:])

            # ---- all-gather the fp16 table (full shard incl. padding) ----
            nc.gpsimd.collective_compute(
                "AllGather",
                mybir.AluOpType.bypass,
                replica_groups=[list(range(N_CORES))],
                ins=[table_pad[:, :].opt()],
                outs=[table_full[:].opt()],
            )

            table_pairs = table_full[:].rearrange("(a two) e -> a (two e)",
                                                  two=2)
            # odd-sized chunk first: it lands in the slow post-collective
            # window and keeps the tail uniform
            chunks = []
            t0 = 0
            if n_tiles % chunk_tiles:
                chunks.append((0, n_tiles % chunk_tiles))
                t0 = n_tiles % chunk_tiles
            while t0 < n_tiles:
                chunks.append((t0, chunk_tiles))
                t0 += chunk_tiles

            # ---- phase 2: pair-gather + attention ----
            for ci, (t0, nt) in enumerate(chunks):
                nidx = nt * K * P
                kxg = kxgp.tile([P, nt * K * PAIR], F16, tag="kxg")
                nc.gpsimd.dma_gather(
                    kxg[:].rearrange("p (s e) -> p s e", e=PAIR),
                    table_pairs,
                    idx_sb[:, ds(t0 * K * 8, nt * K * 8)],
                    nidx, nidx, PAIR,
                    single_packet=False,
                    queue_num=ci % 4,
                )
                kx4 = kxg[:].rearrange("p (t s e) -> p t s e", s=S, e=ROW)
                qc_ap = (q_sb[:, ds(t0 * QC, nt * QC)]
                         .rearrange("p (t e) -> p t e", e=QC)
                         .unsqueeze(2).broadcast_to([P, nt, S, QC]))
                prod = wp.tile([P, nt * S * QC], F16, tag="prod")
                pr5 = prod[:].rearrange("p (t s e) -> p t s e", s=S, e=QC)
                nc.vector.tensor_tensor(out=pr5, in0=kx4[:, :, :, 0:QC],
                                        in1=qc_ap, op=mybir.AluOpType.mult)
                s2a = wp.tile([P, nt * S], F16, tag="s2a")
                with nc.allow_low_precision("scores ~|3| fit fp16"):
                    nc.vector.tensor_reduce(out=s2a[:], in_=pr5,
                                            axis=mybir.AxisListType.X,
                                            op=mybir.AluOpType.add)
                sm2a = wp.tile([P, nt * S], F16, tag="sm2a")
                nc.vector.tensor_tensor(out=sm2a[:], in0=s2a[:],
                                        in1=cm_sb[:, ds(t0 * S, nt * S)],
                                        op=mybir.AluOpType.mult)
                ea2 = wp.tile([P, nt * S], F16, tag="ea2")
                nc.scalar.activation(out=ea2[:], in_=sm2a[:],
                                     func=mybir.ActivationFunctionType.Exp)
                e2m = wp.tile([P, nt * S], F16, tag="e2m")
                nc.vector.tensor_tensor(out=e2m[:], in0=ea2[:],
                                        in1=pm_sb[:, ds(t0 * S, nt * S)],
                                        op=mybir.AluOpType.mult)
                sea = wp.tile([P, nt], F32, tag="sea")
                nc.vector.tensor_reduce(
                    out=sea[:], in_=e2m[:].rearrange("p (t s) -> p t s", s=S),
                    axis=mybir.AxisListType.X, op=mybir.AluOpType.add)
                ra = wp.tile([P, nt], F32, tag="ra")
                nc.vector.reciprocal(out=ra[:], in_=sea[:])
                # e-major layout [p, t, 3, s]: long inner runs for both ops
                wpr = wp.tile([P, nt * 3 * S], F32, tag="wpr")
                nc.vector.tensor_tensor(
                    out=wpr[:].rearrange("p (t e s) -> p t e s", s=S, e=3),
                    in0=kx4[:, :, :, QC:QC + 3]
                        .rearrange("p t s e -> p t e s"),
                    in1=e2m[:].rearrange("p (t s) -> p t s", s=S)
                        .unsqueeze(2).broadcast_to([P, nt, 3, S]),
                    op=mybir.AluOpType.mult)
                wsum = wp.tile([P, nt * 3], F32, tag="wsum")
                nc.vector.tensor_reduce(
                    out=wsum[:],
                    in_=wpr[:].rearrange("p (t e s) -> p t e s", s=S, e=3),
                    axis=mybir.AxisListType.X, op=mybir.AluOpType.add)
                disp = wp.tile([P, nt * 3], F32, tag="disp")
                nc.vector.tensor_tensor(
                    out=disp[:].rearrange("p (t e) -> p t e", e=3),
                    in0=wsum[:].rearrange("p (t e) -> p t e", e=3),
                    in1=ra[:].unsqueeze(2).broadcast_to([P, nt, 3]),
                    op=mybir.AluOpType.mult)
                nc.vector.tensor_tensor(
                    out=out_sb[:, ds(t0 * 3, nt * 3)], in0=disp[:],
                    in1=base_sb[:, ds(t0 * 3, nt * 3)],
                    op=mybir.AluOpType.add)

            nc.sync.dma_start(out=out[:, :], in_=out_sb[:])

    nc.finalize()
    return nc


def prep_inputs(sampled_points, sampled_x, edge_index_filtered,
                W_concat, b_concat, W_out, b_out, W_q, b_q, W_k, b_k,
                n_total, shard, n_tiles):
    """Host-side layout prep + weight folding. Returns in_maps for 8 cores."""
    P = 128
    scale = np.sqrt(np.float32(H // 2), dtype=np.float32) + 1e-6

    Wc = W_concat.astype(np.float64)
    Wo = W_out.astype(np.float64)
    M1 = Wo @ Wc[:, :H]                                    # [3,128]
    M2 = Wo @ Wc[:, H:]                                    # [3,3]
    c0 = b_concat.astype(np.float64) @ Wo.T + b_out.astype(np.float64)

    w1 = np.zeros((P, W1C), np.float64)
    w1[:, 0:64] = W_k.astype(np.float64).T
    w1[:, 64:67] = M1.T
    w1[:, 67:W1C] = W_q.astype(np.float64).T / scale
    w1 = w1.astype(np.float16)

    brow = np.zeros((1, W1C), np.float64)
    brow[0, 0:64] = b_k.astype(np.float64)
    brow[0, 67:W1C] = b_q.astype(np.float64) / scale
    brow_rep = np.repeat(brow.astype(np.float16), P, 0)

    dst = np.asarray(edge_index_filtered[1]).reshape(n_total, K)
    valid = n_total // N_CORES

    # node j -> partition-major table row: owning core r, local (t, p) slot,
    # row = r*shard + p*n_tiles + t
    def row_of(j):
        r_core = j // valid
        loc = j % valid
        return r_core * shard + (loc % P) * n_tiles + loc // P

    in_maps = []
    for r in range(N_CORES):
        rows = slice(r * valid, (r + 1) * valid)
        x_r = np.zeros((shard, H), np.float16)
        x_r[:valid] = sampled_x[rows].astype(np.float16)
        nb_r = np.zeros((shard, K), np.int64)
        nb_r[:valid] = dst[rows]
        pt_r = np.zeros((shard, 3), np.float64)
        pt_r[:valid] = sampled_points[rows].astype(np.float64)

        px = (pt_r @ M2.T).astype(np.float16)
        base = (pt_r + c0[None, :] - pt_r @ M2.T).astype(np.float32)

        def swz(a, width):
            return (a.reshape(n_tiles, P, width).transpose(1, 0, 2)
                    .reshape(P, n_tiles * width).copy())

        # gather indices: position (slot = t*K+k, p) -> idx = row//2, stored
        # int16 wrapped-16: [16, pos//16] replicated to all 8 partition groups
        nbs = nb_r.reshape(n_tiles, P, K)
        npos = n_tiles * K * P
        pos = np.arange(npos)
        slot, p = pos // P, pos % P
        t_, k_ = slot // K, slot % K
        stream = row_of(nbs[t_, p, k_])
        idxw = (stream // 2).astype(np.int16).reshape(-1, 16).T  # [16, npos/16]
        idx_rep = np.tile(idxw, (8, 1))                          # [128, npos/16]

        par = (stream % 2).astype(np.float32)                    # h=1 half
        nz = (nbs[t_, p, k_] != 0).astype(np.float32)
        # masks laid out [p, (t k h)]
        pmask = np.zeros((P, n_tiles * K * 2), np.float32)
        pmask[p, (t_ * K + k_) * 2 + 0] = 1.0 - par
        pmask[p, (t_ * K + k_) * 2 + 1] = par
        cmask = pmask.copy()
        cmask[p, (t_ * K + k_) * 2 + 0] *= nz
        cmask[p, (t_ * K + k_) * 2 + 1] *= nz

        in_maps.append({
            "ax": np.ascontiguousarray(x_r.T),
            "idx16": np.ascontiguousarray(idx_rep),
            "cmask": cmask.astype(np.float32),
            "pmask": pmask.astype(np.float16),
            "w1": w1,
            "brow": brow_rep,
            "pxp": swz(px, 3),
            "basep": swz(base, 3),
        })
    return in_maps


def assemble_output(results, n_total, n_tiles):
    P = 128
    valid = n_total // N_CORES
    outs = []
    for r in range(N_CORES):
        o = results[r]["out"]
        o = (o.reshape(P, n_tiles, 3).transpose(1, 0, 2)
             .reshape(n_tiles * P, 3)[:valid])
        outs.append(o)
    return np.concatenate(outs, axis=0).astype(np.float32)


_CACHED = {}


def _get_program(n_total, shard, n_tiles):
    key = (n_total, shard, n_tiles)
    if key not in _CACHED:
        _CACHED[key] = build_program(n_total, shard, n_tiles)
    return _CACHED[key]


def kernel(sampled_points, sampled_x, edge_index_filtered,
           W_concat, b_concat, W_out, b_out, W_q, b_q, W_k, b_k):
    n_total = 60000
    n_tiles = 59
    shard = n_tiles * 128
    nc = _get_program(n_total, shard, n_tiles)
    in_maps = prep_inputs(
        np.asarray(sampled_points), np.asarray(sampled_x),
        np.asarray(edge_index_filtered),
        np.asarray(W_concat), np.asarray(b_concat),
        np.asarray(W_out), np.asarray(b_out),
        np.asarray(W_q), np.asarray(b_q),
        np.asarray(W_k), np.asarray(b_k),
        n_total, shard, n_tiles)
    res = run_bass_kernel_spmd(nc, in_maps, list(range(N_CORES)))
    return assemble_output(res.results, n_total, n_tiles)


# revision 33
# speedup vs baseline: 2.6827x; 1.0808x over previous
"""Trainium2 Bass kernel for MeshGNN message passing (8 NeuronCores, SPMD).

Math reformulation (exact): since softmax weights sum to 1 and the output MLP is
linear, fold W_concat/W_out into per-node quantities:
    M1 = W_out @ W_concat[:, :128]   [3,128]
    M2 = W_out @ W_concat[:, 128:]   [3,3]
    c0 = b_concat @ W_out.T + b_out  [3]
    kx[j] = x[j] @ W_k.T + b_k                  (64,)   -> table
    w[j]  = x[j] @ M1.T + p[j] @ M2.T           (3,)    -> table
    q[n]  = (x[n] @ W_q.T + b_q) / scale        (64,)
    scores[n,k] = q[n] . kx[nbr]
    e = exp(scores * (nbr != 0))                         (scores bounded ~3)
    out[n] = base[n] + sum_k e_k * w[nbr] / sum_k e_k,
    base = p + c0 - p @ M2.T                             (host-computed)

Per-node table rows of 128 fp16 (=256B): [kx(64)|w(3)|pad61], laid out
partition-major (row = p*n_tiles + t) so the table write is one contiguous
stream. Rows are fetched with dma_gather in PAIRS (512B, idx = row//2 fits
int16; negative int16 idxs are broken on HW so exact row indexing is
impossible). The pair-half select is folded into the softmax:
e2m[s] = pmask[s]*exp(s2a[s]*cmask[s]) is exact for the selected half, 0 for
the wrong half, and exp(0)=1 for padding edges (matching the reference's
softmax(scores*mask) semantics).

Phase 1 computes the own-shard table with one matmul per 128-node tile (3
tiles per PSUM bank) and fused DVE copy+bias; an AllGather shares the fp16
table. Desc-gen on the Q7 SWDGE (~2.7ns/idx, engine-serial) is the floor for
phase 2; dummy warmup gathers at t=0 pull the Q7 dma_gather IRAM overlay into
each queue's CPU pair so the first real chunks run at full rate.
"""

import sys

import numpy as np

sys.path.insert(0, "/opt/trn_rl_repo")

import concourse.bass as bass
import concourse.mybir as mybir
import concourse.tile as tile
from concourse import bacc
from concourse.bass import ds, ts
from concourse.bass_utils import run_bass_kernel_spmd

N_CORES = 8
H = 128
K = 15
DT = mybir.dt
F16 = DT.float16
F32 = DT.float32
I16 = DT.int16

ROW = 128                 # fp16 elems per table row (256B)
PAIR = 2 * ROW            # gather element: two rows (512B)
QC = 64                   # q/k dim
W1C = 131                 # matmul cols: kx(64)|w(3)|q(64)
S = 2 * K                 # pair-expanded slots per node


def build_program(n_total, shard, n_tiles, chunk_tiles=2):
    P = 128
    nc = bacc.Bacc(None, debug=False, num_swdge_queues=4)

    ax = nc.declare_dram_parameter("ax", [P, shard], F16, isOutput=False)  # x.T
    idx16 = nc.declare_dram_parameter("idx16", [P, n_tiles * K * 8], I16,
                                      isOutput=False)
    cmask = nc.declare_dram_parameter("cmask", [P, n_tiles * K * 2], F32,
                                      isOutput=False)
    pmask = nc.declare_dram_parameter("pmask", [P, n_tiles * K * 2], F16,
                                      isOutput=False)
    w1 = nc.declare_dram_parameter("w1", [P, W1C], F16, isOutput=False)
    brow = nc.declare_dram_parameter("brow", [P, W1C], F16, isOutput=False)
    pxp = nc.declare_dram_parameter("pxp", [P, n_tiles * 3], F16,
                                    isOutput=False)
    basep = nc.declare_dram_parameter("basep", [P, n_tiles * 3], F32,
                                      isOutput=False)
    out = nc.declare_dram_parameter("out", [P, n_tiles * 3], F32, isOutput=True)

    with tile.TileContext(nc) as tc:
        with (
            tc.tile_pool(name="persist", bufs=1) as pp,
            tc.tile_pool(name="dram", bufs=1, space="DRAM") as dp,
            tc.tile_pool(name="psum", bufs=4, space="PSUM") as psp,
            tc.tile_pool(name="kxgp", bufs=6) as kxgp,
            tc.tile_pool(name="work", bufs=2) as wp,
        ):
            # ---- persistent SBUF ----
            xT = pp.tile([P, shard], F16)
            idx_sb = pp.tile([P, n_tiles * K * 8], I16)
            cm_sb = pp.tile([P, n_tiles * K * 2], F32)
            pm_sb = pp.tile([P, n_tiles * K * 2], F16)
            w1_sb = pp.tile([P, W1C], F16)
            br_sb = pp.tile([P, W1C], F16)
            px_sb = pp.tile([P, n_tiles * 3], F16)
            base_sb = pp.tile([P, n_tiles * 3], F32)
            q_sb = pp.tile([P, n_tiles * QC], F16)
            out_sb = pp.tile([P, n_tiles * 3], F32)
            tblall_sb = pp.tile([P, n_tiles * ROW], F16)

            # table rows laid out partition-major: DRAM row = p*n_tiles + t,
            # so the table write is a contiguous per-partition stream.
            table_pad = dp.tile([shard, ROW], F16, space="DRAM")
            table_full = dp.tile([N_CORES * shard, ROW], F16, space="DRAM",
                                 addr_space="Shared")

            nc.sync.dma_start(out=xT[:], in_=ax[:, :])
            nc.sync.dma_start(out=idx_sb[:], in_=idx16[:, :])
            nc.sync.dma_start(out=cm_sb[:], in_=cmask[:, :])
            nc.sync.dma_start(out=pm_sb[:], in_=pmask[:, :])
            nc.sync.dma_start(out=w1_sb[:], in_=w1[:, :])
            nc.sync.dma_start(out=br_sb[:], in_=brow[:, :])
            nc.sync.dma_start(out=px_sb[:], in_=pxp[:, :])
            nc.sync.dma_start(out=base_sb[:], in_=basep[:, :])

            nc.vector.memset(tblall_sb[:], 0)

            # ---- phase 1: x@W1 (3 tiles per PSUM bank), DVE copy+bias ----
            tbl3 = tblall_sb[:].rearrange("p (t e) -> p t e", e=ROW)
            q3 = q_sb[:].rearrange("p (t e) -> p t e", e=QC)
            NT = n_tiles
            GROUP = 3
            g0 = 0
            while g0 < n_tiles:
                gn = min(GROUP, n_tiles - g0)
                ps = psp.tile([P, GROUP * W1C], F32, space="PSUM", tag="ps")
                for i in range(gn):
                    nc.tensor.matmul(out=ps[:, ds(i * W1C, W1C)],
                                     lhsT=xT[:, ts(g0 + i, P)], rhs=w1_sb[:],
                                     start=True, stop=True)
                psv = ps[:].rearrange("p (g c) -> p g c", c=W1C)
                nc.vector.tensor_tensor(
                    out=tbl3[:, g0:g0 + gn, 0:67], in0=psv[:, 0:gn, 0:67],
                    in1=br_sb[:, 0:67].unsqueeze(1).broadcast_to([P, gn, 67]),
                    op=mybir.AluOpType.add)
                nc.vector.tensor_tensor(
                    out=q3[:, g0:g0 + gn, :], in0=psv[:, 0:gn, 67:W1C],
                    in1=br_sb[:, 67:W1C].unsqueeze(1).broadcast_to([P, gn, QC]),
                    op=mybir.AluOpType.add)
                g0 += gn
            nc.vector.tensor_tensor(
                out=tbl3[:, :, 64:67], in0=tbl3[:, :, 64:67],
                in1=px_sb[:].rearrange("p (t e) -> p t e", e=3),
                op=mybir.AluOpType.add)

            # one contiguous DMA for the whole table slice
            nc.sync.dma_start(
                out=table_pad[:].rearrange("(p t) e -> p (t e)", p=P),
                in_=tblall_sb[:])

            # ---- all-gather the fp16 table (full shard incl. padding) ----
            nc.gpsimd.collective_compute(
                "AllGather",
                mybir.AluOpType.bypass,
                replica_groups=[list(range(N_CORES))],
                ins=[table_pad[:, :].opt()],
                outs=[table_full[:].opt()],
            )

            table_pairs = table_full[:].rearrange("(a two) e -> a (two e)",
                                                  two=2)
            # odd-sized chunk first: it lands in the slow post-collective
            # window and keeps the tail uniform
            chunks = []
            t0 = 0
            if n_tiles % chunk_tiles:
                chunks.append((0, n_tiles % chunk_tiles))
                t0 = n_tiles % chunk_tiles
            while t0 < n_tiles:
                chunks.append((t0, chunk_tiles))
                t0 += chunk_tiles

            # ---- phase 2: pair-gather + attention ----
            for ci, (t0, nt) in enumerate(chunks):
                nidx = nt * K * P
                kxg = kxgp.tile([P, nt * K * PAIR], F16, tag="kxg")
                nc.gpsimd.dma_gather(
                    kxg[:].rearrange("p (s e) -> p s e", e=PAIR),
                    table_pairs,
                    idx_sb[:, ds(t0 * K * 8, nt * K * 8)],
                    nidx, nidx, PAIR,
                    single_packet=False,
                    queue_num=ci % 4,
                )
                kx4 = kxg[:].rearrange("p (t s e) -> p t s e", s=S, e=ROW)
                qc_ap = (q_sb[:, ds(t0 * QC, nt * QC)]
                         .rearrange("p (t e) -> p t e", e=QC)
                         .unsqueeze(2).broadcast_to([P, nt, S, QC]))
                prod = wp.tile([P, nt * S * QC], F16, tag="prod")
                pr5 = prod[:].rearrange("p (t s e) -> p t s e", s=S, e=QC)
                nc.vector.tensor_tensor(out=pr5, in0=kx4[:, :, :, 0:QC],
                                        in1=qc_ap, op=mybir.AluOpType.mult)
                s2a = wp.tile([P, nt * S], F16, tag="s2a")
                with nc.allow_low_precision("scores ~|3| fit fp16"):
                    nc.vector.tensor_reduce(out=s2a[:], in_=pr5,
                                            axis=mybir.AxisListType.X,
                                            op=mybir.AluOpType.add)
                sm2a = wp.tile([P, nt * S], F16, tag="sm2a")
                nc.vector.tensor_tensor(out=sm2a[:], in0=s2a[:],
                                        in1=cm_sb[:, ds(t0 * S, nt * S)],
                                        op=mybir.AluOpType.mult)
                ea2 = wp.tile([P, nt * S], F16, tag="ea2")
                nc.scalar.activation(out=ea2[:], in_=sm2a[:],
                                     func=mybir.ActivationFunctionType.Exp)
                e2m = wp.tile([P, nt * S], F16, tag="e2m")
                nc.vector.tensor_tensor(out=e2m[:], in0=ea2[:],
                                        in1=pm_sb[:, ds(t0 * S, nt * S)],
                                        op=mybir.AluOpType.mult)
                sea = wp.tile([P, nt], F32, tag="sea")
                nc.vector.tensor_reduce(
                    out=sea[:], in_=e2m[:].rearrange("p (t s) -> p t s", s=S),
                    axis=mybir.AxisListType.X, op=mybir.AluOpType.add)
                ra = wp.tile([P, nt], F32, tag="ra")
                nc.vector.reciprocal(out=ra[:], in_=sea[:])
                wpr = wp.tile([P, nt * S * 3], F32, tag="wpr")
                nc.vector.tensor_tensor(
                    out=wpr[:].rearrange("p (t s e) -> p t s e", s=S, e=3),
                    in0=kx4[:, :, :, QC:QC + 3],
                    in1=e2m[:].rearrange("p (t s) -> p t s", s=S)
                        .unsqueeze(3).broadcast_to([P, nt, S, 3]),
                    op=mybir.AluOpType.mult)
                wsum = wp.tile([P, nt * 3], F32, tag="wsum")
                nc.vector.tensor_reduce(
                    out=wsum[:],
                    in_=wpr[:].rearrange("p (t s e) -> p t e s", s=S, e=3),
                    axis=mybir.AxisListType.X, op=mybir.AluOpType.add)
                disp = wp.tile([P, nt * 3], F32, tag="disp")
                nc.vector.tensor_tensor(
                    out=disp[:].rearrange("p (t e) -> p t e", e=3),
                    in0=wsum[:].rearrange("p (t e) -> p t e", e=3),
                    in1=ra[:].unsqueeze(2).broadcast_to([P, nt, 3]),
                    op=mybir.AluOpType.mult)
                nc.vector.tensor_tensor(
                    out=out_sb[:, ds(t0 * 3, nt * 3)], in0=disp[:],
                    in1=base_sb[:, ds(t0 * 3, nt * 3)],
                    op=mybir.AluOpType.add)

            nc.sync.dma_start(out=out[:, :], in_=out_sb[:])

    nc.finalize()
    return nc


def prep_inputs(sampled_points, sampled_x, edge_index_filtered,
                W_concat, b_concat, W_out, b_out, W_q, b_q, W_k, b_k,
                n_total, shard, n_tiles):
    """Host-side layout prep + weight folding. Returns in_maps for 8 cores."""
    P = 128
    scale = np.sqrt(np.float32(H // 2), dtype=np.float32) + 1e-6

    Wc = W_concat.astype(np.float64)
    Wo = W_out.astype(np.float64)
    M1 = Wo @ Wc[:, :H]                                    # [3,128]
    M2 = Wo @ Wc[:, H:]                                    # [3,3]
    c0 = b_concat.astype(np.float64) @ Wo.T + b_out.astype(np.float64)

    w1 = np.zeros((P, W1C), np.float64)
    w1[:, 0:64] = W_k.astype(np.float64).T
    w1[:, 64:67] = M1.T
    w1[:, 67:W1C] = W_q.astype(np.float64).T / scale
    w1 = w1.astype(np.float16)

    brow = np.zeros((1, W1C), np.float64)
    brow[0, 0:64] = b_k.astype(np.float64)
    brow[0, 67:W1C] = b_q.astype(np.float64) / scale
    brow_rep = np.repeat(brow.astype(np.float16), P, 0)

    dst = np.asarray(edge_index_filtered[1]).reshape(n_total, K)
    valid = n_total // N_CORES

    # node j -> partition-major table row: owning core r, local (t, p) slot,
    # row = r*shard + p*n_tiles + t
    def row_of(j):
        r_core = j // valid
        loc = j % valid
        return r_core * shard + (loc % P) * n_tiles + loc // P

    in_maps = []
    for r in range(N_CORES):
        rows = slice(r * valid, (r + 1) * valid)
        x_r = np.zeros((shard, H), np.float16)
        x_r[:valid] = sampled_x[rows].astype(np.float16)
        nb_r = np.zeros((shard, K), np.int64)
        nb_r[:valid] = dst[rows]
        pt_r = np.zeros((shard, 3), np.float64)
        pt_r[:valid] = sampled_points[rows].astype(np.float64)

        px = (pt_r @ M2.T).astype(np.float16)
        base = (pt_r + c0[None, :] - pt_r @ M2.T).astype(np.float32)

        def swz(a, width):
            return (a.reshape(n_tiles, P, width).transpose(1, 0, 2)
                    .reshape(P, n_tiles * width).copy())

        # gather indices: position (slot = t*K+k, p) -> idx = row//2, stored
        # int16 wrapped-16: [16, pos//16] replicated to all 8 partition groups
        nbs = nb_r.reshape(n_tiles, P, K)
        npos = n_tiles * K * P
        pos = np.arange(npos)
        slot, p = pos // P, pos % P
        t_, k_ = slot // K, slot % K
        stream = row_of(nbs[t_, p, k_])
        idxw = (stream // 2).astype(np.int16).reshape(-1, 16).T  # [16, npos/16]
        idx_rep = np.tile(idxw, (8, 1))                          # [128, npos/16]

        par = (stream % 2).astype(np.float32)                    # h=1 half
        nz = (nbs[t_, p, k_] != 0).astype(np.float32)
        # masks laid out [p, (t k h)]
        pmask = np.zeros((P, n_tiles * K * 2), np.float32)
        pmask[p, (t_ * K + k_) * 2 + 0] = 1.0 - par
        pmask[p, (t_ * K + k_) * 2 + 1] = par
        cmask = pmask.copy()
        cmask[p, (t_ * K + k_) * 2 + 0] *= nz
        cmask[p, (t_ * K + k_) * 2 + 1] *= nz

        in_maps.append({
            "ax": np.ascontiguousarray(x_r.T),
            "idx16": np.ascontiguousarray(idx_rep),
            "cmask": cmask.astype(np.float32),
            "pmask": pmask.astype(np.float16),
            "w1": w1,
            "brow": brow_rep,
            "pxp": swz(px, 3),
            "basep": swz(base, 3),
        })
    return in_maps


def assemble_output(results, n_total, n_tiles):
    P = 128
    valid = n_total // N_CORES
    outs = []
    for r in range(N_CORES):
        o = results[r]["out"]
        o = (o.reshape(P, n_tiles, 3).transpose(1, 0, 2)
             .reshape(n_tiles * P, 3)[:valid])
        outs.append(o)
    return np.concatenate(outs, axis=0).astype(np.float32)


_CACHED = {}


def _get_program(n_total, shard, n_tiles):
    key = (n_total, shard, n_tiles)
    if key not in _CACHED:
        _CACHED[key] = build_program(n_total, shard, n_tiles)
    return _CACHED[key]


def kernel(sampled_points, sampled_x, edge_index_filtered,
           W_concat, b_concat, W_out, b_out, W_q, b_q, W_k, b_k):
    n_total = 60000
    n_tiles = 59
    shard = n_tiles * 128
    nc = _get_program(n_total, shard, n_tiles)
    in_maps = prep_inputs(
        np.asarray(sampled_points), np.asarray(sampled_x),
        np.asarray(edge_index_filtered),
        np.asarray(W_concat), np.asarray(b_concat),
        np.asarray(W_out), np.asarray(b_out),
        np.asarray(W_q), np.asarray(b_q),
        np.asarray(W_k), np.asarray(b_k),
        n_total, shard, n_tiles)
    res = run_bass_kernel_spmd(nc, in_maps, list(range(N_CORES)))
    return assemble_output(res.results, n_total, n_tiles)


# revision 35
# speedup vs baseline: 2.7229x; 1.0150x over previous
"""Trainium2 Bass kernel for MeshGNN message passing (8 NeuronCores, SPMD).

Math reformulation (exact): since softmax weights sum to 1 and the output MLP is
linear, fold W_concat/W_out into per-node quantities:
    M1 = W_out @ W_concat[:, :128]   [3,128]
    M2 = W_out @ W_concat[:, 128:]   [3,3]
    c0 = b_concat @ W_out.T + b_out  [3]
    kx[j] = x[j] @ W_k.T + b_k                  (64,)   -> table
    w[j]  = x[j] @ M1.T + p[j] @ M2.T           (3,)    -> table
    q[n]  = (x[n] @ W_q.T + b_q) / scale        (64,)
    scores[n,k] = q[n] . kx[nbr]
    e = exp(scores * (nbr != 0))                         (scores bounded ~3)
    out[n] = base[n] + sum_k e_k * w[nbr] / sum_k e_k,
    base = p + c0 - p @ M2.T                             (host-computed)

Per-node table rows of 128 fp16 (=256B): [kx(64)|w(3)|pad61], laid out
partition-major (row = p*n_tiles + t) so the table write is one contiguous
stream. Rows are fetched with dma_gather in PAIRS (512B, idx = row//2 fits
int16; negative int16 idxs are broken on HW so exact row indexing is
impossible). The pair-half select is folded into the softmax:
e2m[s] = pmask[s]*exp(s2a[s]*cmask[s]) is exact for the selected half, 0 for
the wrong half, and exp(0)=1 for padding edges (matching the reference's
softmax(scores*mask) semantics).

Phase 1 computes the own-shard table with one matmul per 128-node tile (3
tiles per PSUM bank) and fused DVE copy+bias; an AllGather shares the fp16
table. Desc-gen on the Q7 SWDGE (~2.7ns/idx, engine-serial) is the floor for
phase 2; 2-tile chunks (3840 idx) rotate across the 4 SWDGE queues so the
descriptor ring never drains mid-instruction.
"""

import sys

import numpy as np

sys.path.insert(0, "/opt/trn_rl_repo")

import concourse.bass as bass
import concourse.mybir as mybir
import concourse.tile as tile
from concourse import bacc
from concourse.bass import ds, ts
from concourse.bass_utils import run_bass_kernel_spmd

N_CORES = 8
H = 128
K = 15
DT = mybir.dt
F16 = DT.float16
F32 = DT.float32
I16 = DT.int16

ROW = 128                 # fp16 elems per table row (256B)
PAIR = 2 * ROW            # gather element: two rows (512B)
QC = 64                   # q/k dim
W1C = 131                 # matmul cols: kx(64)|w(3)|q(64)
S = 2 * K                 # pair-expanded slots per node


def build_program(n_total, shard, n_tiles, chunk_tiles=2):
    P = 128
    nc = bacc.Bacc(None, debug=False, num_swdge_queues=4)

    ax = nc.declare_dram_parameter("ax", [P, shard], F16, isOutput=False)  # x.T
    idx16 = nc.declare_dram_parameter("idx16", [P, n_tiles * K * 8], I16,
                                      isOutput=False)
    cmask = nc.declare_dram_parameter("cmask", [P, n_tiles * K * 2], F32,
                                      isOutput=False)
    pmask = nc.declare_dram_parameter("pmask", [P, n_tiles * K * 2], F16,
                                      isOutput=False)
    w1 = nc.declare_dram_parameter("w1", [P, W1C], F16, isOutput=False)
    brow = nc.declare_dram_parameter("brow", [P, W1C], F16, isOutput=False)
    pxp = nc.declare_dram_parameter("pxp", [P, n_tiles * 3], F16,
                                    isOutput=False)
    basep = nc.declare_dram_parameter("basep", [P, n_tiles * 3], F32,
                                      isOutput=False)
    out = nc.declare_dram_parameter("out", [P, n_tiles * 3], F32, isOutput=True)

    with tile.TileContext(nc) as tc:
        with (
            tc.tile_pool(name="persist", bufs=1) as pp,
            tc.tile_pool(name="dram", bufs=1, space="DRAM") as dp,
            tc.tile_pool(name="psum", bufs=4, space="PSUM") as psp,
            tc.tile_pool(name="kxgp", bufs=6) as kxgp,
            tc.tile_pool(name="work", bufs=2) as wp,
        ):
            # ---- persistent SBUF ----
            xT = pp.tile([P, shard], F16)
            idx_sb = pp.tile([P, n_tiles * K * 8], I16)
            cm_sb = pp.tile([P, n_tiles * K * 2], F32)
            pm_sb = pp.tile([P, n_tiles * K * 2], F16)
            w1_sb = pp.tile([P, W1C], F16)
            br_sb = pp.tile([P, W1C], F16)
            px_sb = pp.tile([P, n_tiles * 3], F16)
            base_sb = pp.tile([P, n_tiles * 3], F32)
            q_sb = pp.tile([P, n_tiles * QC], F16)
            out_sb = pp.tile([P, n_tiles * 3], F32)
            tblall_sb = pp.tile([P, n_tiles * ROW], F16)

            # table rows laid out partition-major: DRAM row = p*n_tiles + t,
            # so the table write is a contiguous per-partition stream.
            table_pad = dp.tile([shard, ROW], F16, space="DRAM")
            table_full = dp.tile([N_CORES * shard, ROW], F16, space="DRAM",
                                 addr_space="Shared")

            nc.sync.dma_start(out=xT[:], in_=ax[:, :])
            nc.sync.dma_start(out=idx_sb[:], in_=idx16[:, :])
            nc.sync.dma_start(out=cm_sb[:], in_=cmask[:, :])
            nc.sync.dma_start(out=pm_sb[:], in_=pmask[:, :])
            nc.sync.dma_start(out=w1_sb[:], in_=w1[:, :])
            nc.sync.dma_start(out=br_sb[:], in_=brow[:, :])
            nc.sync.dma_start(out=px_sb[:], in_=pxp[:, :])
            nc.sync.dma_start(out=base_sb[:], in_=basep[:, :])

            nc.vector.memset(tblall_sb[:], 0)

            # ---- phase 1: x@W1 (3 tiles per PSUM bank), DVE copy+bias ----
            tbl3 = tblall_sb[:].rearrange("p (t e) -> p t e", e=ROW)
            q3 = q_sb[:].rearrange("p (t e) -> p t e", e=QC)
            NT = n_tiles
            GROUP = 3
            g0 = 0
            while g0 < n_tiles:
                gn = min(GROUP, n_tiles - g0)
                ps = psp.tile([P, GROUP * W1C], F32, space="PSUM", tag="ps")
                for i in range(gn):
                    nc.tensor.matmul(out=ps[:, ds(i * W1C, W1C)],
                                     lhsT=xT[:, ts(g0 + i, P)], rhs=w1_sb[:],
                                     start=True, stop=True)
                psv = ps[:].rearrange("p (g c) -> p g c", c=W1C)
                nc.vector.tensor_tensor(
                    out=tbl3[:, g0:g0 + gn, 0:67], in0=psv[:, 0:gn, 0:67],
                    in1=br_sb[:, 0:67].unsqueeze(1).broadcast_to([P, gn, 67]),
                    op=mybir.AluOpType.add)
                nc.vector.tensor_tensor(
                    out=q3[:, g0:g0 + gn, :], in0=psv[:, 0:gn, 67:W1C],
                    in1=br_sb[:, 67:W1C].unsqueeze(1).broadcast_to([P, gn, QC]),
                    op=mybir.AluOpType.add)
                g0 += gn
            nc.vector.tensor_tensor(
                out=tbl3[:, :, 64:67], in0=tbl3[:, :, 64:67],
                in1=px_sb[:].rearrange("p (t e) -> p t e", e=3),
                op=mybir.AluOpType.add)

            # one contiguous DMA for the whole table slice
            nc.sync.dma_start(
                out=table_pad[:].rearrange("(p t) e -> p (t e)", p=P),
                in_=tblall_sb[:])

            # ---- all-gather the fp16 table (full shard incl. padding) ----
            nc.gpsimd.collective_compute(
                "AllGather",
                mybir.AluOpType.bypass,
                replica_groups=[list(range(N_CORES))],
                ins=[table_pad[:, :].opt()],
                outs=[table_full[:].opt()],
            )

            table_pairs = table_full[:].rearrange("(a two) e -> a (two e)",
                                                  two=2)
            # odd-sized chunk first: it lands in the slow post-collective
            # window and keeps the tail uniform
            chunks = []
            t0 = 0
            if n_tiles % chunk_tiles:
                chunks.append((0, n_tiles % chunk_tiles))
                t0 = n_tiles % chunk_tiles
            while t0 < n_tiles:
                chunks.append((t0, chunk_tiles))
                t0 += chunk_tiles

            # ---- phase 2: pair-gather + attention ----
            for ci, (t0, nt) in enumerate(chunks):
                nidx = nt * K * P
                kxg = kxgp.tile([P, nt * K * PAIR], F16, tag="kxg")
                nc.gpsimd.dma_gather(
                    kxg[:].rearrange("p (s e) -> p s e", e=PAIR),
                    table_pairs,
                    idx_sb[:, ds(t0 * K * 8, nt * K * 8)],
                    nidx, nidx, PAIR,
                    single_packet=False,
                    queue_num=ci % 4,
                )
                kx4 = kxg[:].rearrange("p (t s e) -> p t s e", s=S, e=ROW)
                qc_ap = (q_sb[:, ds(t0 * QC, nt * QC)]
                         .rearrange("p (t e) -> p t e", e=QC)
                         .unsqueeze(2).broadcast_to([P, nt, S, QC]))
                prod = wp.tile([P, nt * S * QC], F16, tag="prod")
                pr5 = prod[:].rearrange("p (t s e) -> p t s e", s=S, e=QC)
                nc.vector.tensor_tensor(out=pr5, in0=kx4[:, :, :, 0:QC],
                                        in1=qc_ap, op=mybir.AluOpType.mult)
                s2a = wp.tile([P, nt * S], F16, tag="s2a")
                with nc.allow_low_precision("scores ~|3| fit fp16"):
                    nc.vector.tensor_reduce(out=s2a[:], in_=pr5,
                                            axis=mybir.AxisListType.X,
                                            op=mybir.AluOpType.add)
                sm2a = wp.tile([P, nt * S], F16, tag="sm2a")
                nc.vector.tensor_tensor(out=sm2a[:], in0=s2a[:],
                                        in1=cm_sb[:, ds(t0 * S, nt * S)],
                                        op=mybir.AluOpType.mult)
                ea2 = wp.tile([P, nt * S], F16, tag="ea2")
                nc.scalar.activation(out=ea2[:], in_=sm2a[:],
                                     func=mybir.ActivationFunctionType.Exp)
                e2m = wp.tile([P, nt * S], F16, tag="e2m")
                nc.vector.tensor_tensor(out=e2m[:], in0=ea2[:],
                                        in1=pm_sb[:, ds(t0 * S, nt * S)],
                                        op=mybir.AluOpType.mult)
                sea = wp.tile([P, nt], F32, tag="sea")
                nc.vector.tensor_reduce(
                    out=sea[:], in_=e2m[:].rearrange("p (t s) -> p t s", s=S),
                    axis=mybir.AxisListType.X, op=mybir.AluOpType.add)
                ra = wp.tile([P, nt], F32, tag="ra")
                nc.vector.reciprocal(out=ra[:], in_=sea[:])
                wpr = wp.tile([P, nt * S * 3], F32, tag="wpr")
                nc.vector.tensor_tensor(
                    out=wpr[:].rearrange("p (t s e) -> p t s e", s=S, e=3),
                    in0=kx4[:, :, :, QC:QC + 3],
                    in1=e2m[:].rearrange("p (t s) -> p t s", s=S)
                        .unsqueeze(3).broadcast_to([P, nt, S, 3]),
                    op=mybir.AluOpType.mult)
                wsum = wp.tile([P, nt * 3], F32, tag="wsum")
                nc.vector.tensor_reduce(
                    out=wsum[:],
                    in_=wpr[:].rearrange("p (t s e) -> p t e s", s=S, e=3),
                    axis=mybir.AxisListType.X, op=mybir.AluOpType.add)
                disp = wp.tile([P, nt * 3], F32, tag="disp")
                nc.vector.tensor_tensor(
                    out=disp[:].rearrange("p (t e) -> p t e", e=3),
                    in0=wsum[:].rearrange("p (t e) -> p t e", e=3),
                    in1=ra[:].unsqueeze(2).broadcast_to([P, nt, 3]),
                    op=mybir.AluOpType.mult)
                nc.vector.tensor_tensor(
                    out=out_sb[:, ds(t0 * 3, nt * 3)], in0=disp[:],
                    in1=base_sb[:, ds(t0 * 3, nt * 3)],
                    op=mybir.AluOpType.add)

            nc.sync.dma_start(out=out[:, :], in_=out_sb[:])

    nc.finalize()
    return nc


def prep_inputs(sampled_points, sampled_x, edge_index_filtered,
                W_concat, b_concat, W_out, b_out, W_q, b_q, W_k, b_k,
                n_total, shard, n_tiles):
    """Host-side layout prep + weight folding. Returns in_maps for 8 cores."""
    P = 128
    scale = np.sqrt(np.float32(H // 2), dtype=np.float32) + 1e-6

    Wc = W_concat.astype(np.float64)
    Wo = W_out.astype(np.float64)
    M1 = Wo @ Wc[:, :H]                                    # [3,128]
    M2 = Wo @ Wc[:, H:]                                    # [3,3]
    c0 = b_concat.astype(np.float64) @ Wo.T + b_out.astype(np.float64)

    w1 = np.zeros((P, W1C), np.float64)
    w1[:, 0:64] = W_k.astype(np.float64).T
    w1[:, 64:67] = M1.T
    w1[:, 67:W1C] = W_q.astype(np.float64).T / scale
    w1 = w1.astype(np.float16)

    brow = np.zeros((1, W1C), np.float64)
    brow[0, 0:64] = b_k.astype(np.float64)
    brow[0, 67:W1C] = b_q.astype(np.float64) / scale
    brow_rep = np.repeat(brow.astype(np.float16), P, 0)

    dst = np.asarray(edge_index_filtered[1]).reshape(n_total, K)
    valid = n_total // N_CORES

    # node j -> partition-major table row: owning core r, local (t, p) slot,
    # row = r*shard + p*n_tiles + t
    def row_of(j):
        r_core = j // valid
        loc = j % valid
        return r_core * shard + (loc % P) * n_tiles + loc // P

    in_maps = []
    for r in range(N_CORES):
        rows = slice(r * valid, (r + 1) * valid)
        x_r = np.zeros((shard, H), np.float16)
        x_r[:valid] = sampled_x[rows].astype(np.float16)
        nb_r = np.zeros((shard, K), np.int64)
        nb_r[:valid] = dst[rows]
        pt_r = np.zeros((shard, 3), np.float64)
        pt_r[:valid] = sampled_points[rows].astype(np.float64)

        px = (pt_r @ M2.T).astype(np.float16)
        base = (pt_r + c0[None, :] - pt_r @ M2.T).astype(np.float32)

        def swz(a, width):
            return (a.reshape(n_tiles, P, width).transpose(1, 0, 2)
                    .reshape(P, n_tiles * width).copy())

        # gather indices: position (slot = t*K+k, p) -> idx = row//2, stored
        # int16 wrapped-16: [16, pos//16] replicated to all 8 partition groups
        nbs = nb_r.reshape(n_tiles, P, K)
        npos = n_tiles * K * P
        pos = np.arange(npos)
        slot, p = pos // P, pos % P
        t_, k_ = slot // K, slot % K
        stream = row_of(nbs[t_, p, k_])
        idxw = (stream // 2).astype(np.int16).reshape(-1, 16).T  # [16, npos/16]
        idx_rep = np.tile(idxw, (8, 1))                          # [128, npos/16]

        par = (stream % 2).astype(np.float32)                    # h=1 half
        nz = (nbs[t_, p, k_] != 0).astype(np.float32)
        # masks laid out [p, (t k h)]
        pmask = np.zeros((P, n_tiles * K * 2), np.float32)
        pmask[p, (t_ * K + k_) * 2 + 0] = 1.0 - par
        pmask[p, (t_ * K + k_) * 2 + 1] = par
        cmask = pmask.copy()
        cmask[p, (t_ * K + k_) * 2 + 0] *= nz
        cmask[p, (t_ * K + k_) * 2 + 1] *= nz

        in_maps.append({
            "ax": np.ascontiguousarray(x_r.T),
            "idx16": np.ascontiguousarray(idx_rep),
            "cmask": cmask.astype(np.float32),
            "pmask": pmask.astype(np.float16),
            "w1": w1,
            "brow": brow_rep,
            "pxp": swz(px, 3),
            "basep": swz(base, 3),
        })
    return in_maps


def assemble_output(results, n_total, n_tiles):
    P = 128
    valid = n_total // N_CORES
    outs = []
    for r in range(N_CORES):
        o = results[r]["out"]
        o = (o.reshape(P, n_tiles, 3).transpose(1, 0, 2)
             .reshape(n_tiles * P, 3)[:valid])
        outs.append(o)
    return np.concatenate(outs, axis=0).astype(np.float32)


_CACHED = {}


def _get_program(n_total, shard, n_tiles):
    key = (n_total, shard, n_tiles)
    if key not in _CACHED:
        _CACHED[key] = build_program(n_total, shard, n_tiles)
    return _CACHED[key]


def kernel(sampled_points, sampled_x, edge_index_filtered,
           W_concat, b_concat, W_out, b_out, W_q, b_q, W_k, b_k):
    n_total = 60000
    n_tiles = 59
    shard = n_tiles * 128
    nc = _get_program(n_total, shard, n_tiles)
    in_maps = prep_inputs(
        np.asarray(sampled_points), np.asarray(sampled_x),
        np.asarray(edge_index_filtered),
        np.asarray(W_concat), np.asarray(b_concat),
        np.asarray(W_out), np.asarray(b_out),
        np.asarray(W_q), np.asarray(b_q),
        np.asarray(W_k), np.asarray(b_k),
        n_total, shard, n_tiles)
    res = run_bass_kernel_spmd(nc, in_maps, list(range(N_CORES)))
    return assemble_output(res.results, n_total, n_tiles)


# revision 43
# speedup vs baseline: 2.7966x; 1.0271x over previous
"""Trainium2 Bass kernel for MeshGNN message passing (8 NeuronCores, SPMD).

Math reformulation (exact): since softmax weights sum to 1 and the output MLP is
linear, fold W_concat/W_out into per-node quantities:
    M1 = W_out @ W_concat[:, :128]   [3,128]
    M2 = W_out @ W_concat[:, 128:]   [3,3]
    c0 = b_concat @ W_out.T + b_out  [3]
    kx[j] = x[j] @ W_k.T + b_k                  (64,)   -> table
    w[j]  = x[j] @ M1.T + p[j] @ M2.T           (3,)    -> table
    q[n]  = (x[n] @ W_q.T + b_q) / scale        (64,)
    scores[n,k] = q[n] . kx[nbr]
    e = exp(scores * (nbr != 0))                         (scores bounded ~3)
    out[n] = base[n] + sum_k e_k * w[nbr] / sum_k e_k,
    base = p + c0 - p @ M2.T                             (host-computed)

Per-node table rows of 128 fp16 (=256B): [kx(64)|w(3)|pad61], laid out
partition-major (row = p*n_tiles + t) so the table write is one contiguous
stream. Rows are fetched with dma_gather in PAIRS (512B, idx = row//2 fits
int16; negative int16 idxs are broken on HW so exact row indexing is
impossible). The pair-half select is folded into the softmax:
e2m[s] = pmask[s]*exp(s2a[s]*cmask[s]) is exact for the selected half, 0 for
the wrong half, and exp(0)=1 for padding edges (matching the reference's
softmax(scores*mask) semantics).

Phase 1 computes the own-shard table with one matmul per 128-node tile (3
tiles per PSUM bank) and fused DVE copy+bias; an AllGather shares the fp16
table. Desc-gen on the Q7 SWDGE (~2.7ns/idx, engine-serial) is the floor for
phase 2; 2-tile chunks (3840 idx) rotate across the 4 SWDGE queues so the
descriptor ring never drains mid-instruction.
"""

import sys

import numpy as np

sys.path.insert(0, "/opt/trn_rl_repo")

import concourse.bass as bass
import concourse.mybir as mybir
import concourse.tile as tile
from concourse import bacc
from concourse.bass import ds, ts
from concourse.bass_utils import run_bass_kernel_spmd

N_CORES = 8
H = 128
K = 15
DT = mybir.dt
F16 = DT.float16
F32 = DT.float32
I16 = DT.int16

ROW = 128                 # fp16 elems per table row (256B)
PAIR = 2 * ROW            # gather element: two rows (512B)
QC = 64                   # q/k dim
W1C = 131                 # matmul cols: kx(64)|w(3)|q(64)
S = 2 * K                 # pair-expanded slots per node


def build_program(n_total, shard, n_tiles, chunk_tiles=2):
    P = 128
    nc = bacc.Bacc(None, debug=False, num_swdge_queues=4)

    ax = nc.declare_dram_parameter("ax", [P, shard], F16, isOutput=False)  # x.T
    idx16 = nc.declare_dram_parameter("idx16", [P, n_tiles * K * 8], I16,
                                      isOutput=False)
    cmask = nc.declare_dram_parameter("cmask", [P, n_tiles * K * 2], F16,
                                      isOutput=False)
    pmask = nc.declare_dram_parameter("pmask", [P, n_tiles * K * 2], F16,
                                      isOutput=False)
    w1 = nc.declare_dram_parameter("w1", [P, W1C], F16, isOutput=False)
    brow = nc.declare_dram_parameter("brow", [P, W1C], F16, isOutput=False)
    pxp = nc.declare_dram_parameter("pxp", [P, n_tiles * 3], F16,
                                    isOutput=False)
    basep = nc.declare_dram_parameter("basep", [P, n_tiles * 3], F32,
                                      isOutput=False)
    out = nc.declare_dram_parameter("out", [P, n_tiles * 3], F32, isOutput=True)

    with tile.TileContext(nc) as tc:
        with (
            tc.tile_pool(name="persist", bufs=1) as pp,
            tc.tile_pool(name="dram", bufs=1, space="DRAM") as dp,
            tc.tile_pool(name="psum", bufs=4, space="PSUM") as psp,
            tc.tile_pool(name="kxgp", bufs=7) as kxgp,
            tc.tile_pool(name="work", bufs=2) as wp,
        ):
            # ---- persistent SBUF ----
            xT = pp.tile([P, shard], F16)
            idx_sb = pp.tile([P, n_tiles * K * 8], I16)
            cm_sb = pp.tile([P, n_tiles * K * 2], F16)
            pm_sb = pp.tile([P, n_tiles * K * 2], F16)
            w1_sb = pp.tile([P, W1C], F16)
            br_sb = pp.tile([P, W1C], F16)
            px_sb = pp.tile([P, n_tiles * 3], F16)
            base_sb = pp.tile([P, n_tiles * 3], F32)
            q_sb = pp.tile([P, n_tiles * QC], F16)
            out_sb = pp.tile([P, n_tiles * 3], F32)
            tblall_sb = pp.tile([P, n_tiles * ROW], F16)

            # table rows laid out partition-major: DRAM row = p*n_tiles + t,
            # so the table write is a contiguous per-partition stream.
            table_pad = dp.tile([shard, ROW], F16, space="DRAM")
            table_full = dp.tile([N_CORES * shard, ROW], F16, space="DRAM",
                                 addr_space="Shared")

            # phase-1-critical loads on SP; phase-2-only loads on the ACT/DVE
            # HWDGE queues so they don't delay the matmul start
            nc.sync.dma_start(out=xT[:], in_=ax[:, :])
            nc.sync.dma_start(out=w1_sb[:], in_=w1[:, :])
            nc.sync.dma_start(out=br_sb[:], in_=brow[:, :])
            nc.sync.dma_start(out=px_sb[:], in_=pxp[:, :])
            nc.scalar.dma_start(out=idx_sb[:], in_=idx16[:, :])
            nc.scalar.dma_start(out=cm_sb[:], in_=cmask[:, :])
            nc.scalar.dma_start(out=pm_sb[:], in_=pmask[:, :])
            nc.scalar.dma_start(out=base_sb[:], in_=basep[:, :])

            nc.vector.memset(tblall_sb[:], 0)

            # ---- phase 1: x@W1 (3 tiles per PSUM bank), DVE copy+bias ----
            tbl3 = tblall_sb[:].rearrange("p (t e) -> p t e", e=ROW)
            q3 = q_sb[:].rearrange("p (t e) -> p t e", e=QC)
            NT = n_tiles
            GROUP = 3
            g0 = 0
            while g0 < n_tiles:
                gn = min(GROUP, n_tiles - g0)
                ps = psp.tile([P, GROUP * W1C], F32, space="PSUM", tag="ps")
                for i in range(gn):
                    nc.tensor.matmul(out=ps[:, ds(i * W1C, W1C)],
                                     lhsT=xT[:, ts(g0 + i, P)], rhs=w1_sb[:],
                                     start=True, stop=True)
                psv = ps[:].rearrange("p (g c) -> p g c", c=W1C)
                nc.vector.tensor_tensor(
                    out=tbl3[:, g0:g0 + gn, 0:67], in0=psv[:, 0:gn, 0:67],
                    in1=br_sb[:, 0:67].unsqueeze(1).broadcast_to([P, gn, 67]),
                    op=mybir.AluOpType.add)
                nc.vector.tensor_tensor(
                    out=q3[:, g0:g0 + gn, :], in0=psv[:, 0:gn, 67:W1C],
                    in1=br_sb[:, 67:W1C].unsqueeze(1).broadcast_to([P, gn, QC]),
                    op=mybir.AluOpType.add)
                g0 += gn
            nc.vector.tensor_tensor(
                out=tbl3[:, :, 64:67], in0=tbl3[:, :, 64:67],
                in1=px_sb[:].rearrange("p (t e) -> p t e", e=3),
                op=mybir.AluOpType.add)

            # one contiguous DMA for the whole table slice
            nc.sync.dma_start(
                out=table_pad[:].rearrange("(p t) e -> p (t e)", p=P),
                in_=tblall_sb[:])

            # ---- all-gather the fp16 table (full shard incl. padding) ----
            nc.gpsimd.collective_compute(
                "AllGather",
                mybir.AluOpType.bypass,
                replica_groups=[list(range(N_CORES))],
                ins=[table_pad[:, :].opt()],
                outs=[table_full[:].opt()],
            )

            table_pairs = table_full[:].rearrange("(a two) e -> a (two e)",
                                                  two=2)
            # odd-sized chunk first: it lands in the slow post-collective
            # window and keeps the tail uniform
            chunks = []
            t0 = 0
            if n_tiles % chunk_tiles:
                chunks.append((0, n_tiles % chunk_tiles))
                t0 = n_tiles % chunk_tiles
            while t0 < n_tiles:
                chunks.append((t0, chunk_tiles))
                t0 += chunk_tiles

            # ---- phase 2: pair-gather + attention ----
            for ci, (t0, nt) in enumerate(chunks):
                nidx = nt * K * P
                kxg = kxgp.tile([P, nt * K * PAIR], F16, tag="kxg")
                nc.gpsimd.dma_gather(
                    kxg[:].rearrange("p (s e) -> p s e", e=PAIR),
                    table_pairs,
                    idx_sb[:, ds(t0 * K * 8, nt * K * 8)],
                    nidx, nidx, PAIR,
                    single_packet=False,
                    queue_num=ci % 4,
                )
                kx4 = kxg[:].rearrange("p (t s e) -> p t s e", s=S, e=ROW)
                qc_ap = (q_sb[:, ds(t0 * QC, nt * QC)]
                         .rearrange("p (t e) -> p t e", e=QC)
                         .unsqueeze(2).broadcast_to([P, nt, S, QC]))
                prod = wp.tile([P, nt * S * QC], F16, tag="prod")
                pr5 = prod[:].rearrange("p (t s e) -> p t s e", s=S, e=QC)
                nc.vector.tensor_tensor(out=pr5, in0=kx4[:, :, :, 0:QC],
                                        in1=qc_ap, op=mybir.AluOpType.mult)
                s2a = wp.tile([P, nt * S], F16, tag="s2a")
                with nc.allow_low_precision("scores ~|3| fit fp16"):
                    nc.vector.tensor_reduce(out=s2a[:], in_=pr5,
                                            axis=mybir.AxisListType.X,
                                            op=mybir.AluOpType.add)
                sm2a = wp.tile([P, nt * S], F16, tag="sm2a")
                nc.vector.tensor_tensor(out=sm2a[:], in0=s2a[:],
                                        in1=cm_sb[:, ds(t0 * S, nt * S)],
                                        op=mybir.AluOpType.mult)
                ea2 = wp.tile([P, nt * S], F16, tag="ea2")
                nc.scalar.activation(out=ea2[:], in_=sm2a[:],
                                     func=mybir.ActivationFunctionType.Exp)
                e2m = wp.tile([P, nt * S], F16, tag="e2m")
                nc.vector.tensor_tensor(out=e2m[:], in0=ea2[:],
                                        in1=pm_sb[:, ds(t0 * S, nt * S)],
                                        op=mybir.AluOpType.mult)
                sea = wp.tile([P, nt], F32, tag="sea")
                nc.vector.tensor_reduce(
                    out=sea[:], in_=e2m[:].rearrange("p (t s) -> p t s", s=S),
                    axis=mybir.AxisListType.X, op=mybir.AluOpType.add)
                ra = wp.tile([P, nt], F32, tag="ra")
                nc.vector.reciprocal(out=ra[:], in_=sea[:])
                wpr = wp.tile([P, nt * S * 3], F32, tag="wpr")
                nc.vector.tensor_tensor(
                    out=wpr[:].rearrange("p (t s e) -> p t s e", s=S, e=3),
                    in0=kx4[:, :, :, QC:QC + 3],
                    in1=e2m[:].rearrange("p (t s) -> p t s", s=S)
                        .unsqueeze(3).broadcast_to([P, nt, S, 3]),
                    op=mybir.AluOpType.mult)
                wsum = wp.tile([P, nt * 3], F32, tag="wsum")
                nc.vector.tensor_reduce(
                    out=wsum[:],
                    in_=wpr[:].rearrange("p (t s e) -> p t e s", s=S, e=3),
                    axis=mybir.AxisListType.X, op=mybir.AluOpType.add)
                disp = wp.tile([P, nt * 3], F32, tag="disp")
                nc.vector.tensor_tensor(
                    out=disp[:].rearrange("p (t e) -> p t e", e=3),
                    in0=wsum[:].rearrange("p (t e) -> p t e", e=3),
                    in1=ra[:].unsqueeze(2).broadcast_to([P, nt, 3]),
                    op=mybir.AluOpType.mult)
                nc.vector.tensor_tensor(
                    out=out_sb[:, ds(t0 * 3, nt * 3)], in0=disp[:],
                    in1=base_sb[:, ds(t0 * 3, nt * 3)],
                    op=mybir.AluOpType.add)

            nc.sync.dma_start(out=out[:, :], in_=out_sb[:])

    nc.finalize()
    return nc


def prep_inputs(sampled_points, sampled_x, edge_index_filtered,
                W_concat, b_concat, W_out, b_out, W_q, b_q, W_k, b_k,
                n_total, shard, n_tiles):
    """Host-side layout prep + weight folding. Returns in_maps for 8 cores."""
    P = 128
    scale = np.sqrt(np.float32(H // 2), dtype=np.float32) + 1e-6

    Wc = W_concat.astype(np.float64)
    Wo = W_out.astype(np.float64)
    M1 = Wo @ Wc[:, :H]                                    # [3,128]
    M2 = Wo @ Wc[:, H:]                                    # [3,3]
    c0 = b_concat.astype(np.float64) @ Wo.T + b_out.astype(np.float64)

    w1 = np.zeros((P, W1C), np.float64)
    w1[:, 0:64] = W_k.astype(np.float64).T
    w1[:, 64:67] = M1.T
    w1[:, 67:W1C] = W_q.astype(np.float64).T / scale
    w1 = w1.astype(np.float16)

    brow = np.zeros((1, W1C), np.float64)
    brow[0, 0:64] = b_k.astype(np.float64)
    brow[0, 67:W1C] = b_q.astype(np.float64) / scale
    brow_rep = np.repeat(brow.astype(np.float16), P, 0)

    dst = np.asarray(edge_index_filtered[1]).reshape(n_total, K)
    valid = n_total // N_CORES

    # node j -> partition-major table row: owning core r, local (t, p) slot,
    # row = r*shard + p*n_tiles + t
    def row_of(j):
        r_core = j // valid
        loc = j % valid
        return r_core * shard + (loc % P) * n_tiles + loc // P

    in_maps = []
    for r in range(N_CORES):
        rows = slice(r * valid, (r + 1) * valid)
        x_r = np.zeros((shard, H), np.float16)
        x_r[:valid] = sampled_x[rows].astype(np.float16)
        nb_r = np.zeros((shard, K), np.int64)
        nb_r[:valid] = dst[rows]
        pt_r = np.zeros((shard, 3), np.float64)
        pt_r[:valid] = sampled_points[rows].astype(np.float64)

        px = (pt_r @ M2.T).astype(np.float16)
        base = (pt_r + c0[None, :] - pt_r @ M2.T).astype(np.float32)

        def swz(a, width):
            return (a.reshape(n_tiles, P, width).transpose(1, 0, 2)
                    .reshape(P, n_tiles * width).copy())

        # gather indices: position (slot = t*K+k, p) -> idx = row//2, stored
        # int16 wrapped-16: [16, pos//16] replicated to all 8 partition groups
        nbs = nb_r.reshape(n_tiles, P, K)
        npos = n_tiles * K * P
        pos = np.arange(npos)
        slot, p = pos // P, pos % P
        t_, k_ = slot // K, slot % K
        stream = row_of(nbs[t_, p, k_])
        idxw = (stream // 2).astype(np.int16).reshape(-1, 16).T  # [16, npos/16]
        idx_rep = np.tile(idxw, (8, 1))                          # [128, npos/16]

        par = (stream % 2).astype(np.float32)                    # h=1 half
        nz = (nbs[t_, p, k_] != 0).astype(np.float32)
        # masks laid out [p, (t k h)]
        pmask = np.zeros((P, n_tiles * K * 2), np.float32)
        pmask[p, (t_ * K + k_) * 2 + 0] = 1.0 - par
        pmask[p, (t_ * K + k_) * 2 + 1] = par
        cmask = pmask.copy()
        cmask[p, (t_ * K + k_) * 2 + 0] *= nz
        cmask[p, (t_ * K + k_) * 2 + 1] *= nz

        in_maps.append({
            "ax": np.ascontiguousarray(x_r.T),
            "idx16": np.ascontiguousarray(idx_rep),
            "cmask": cmask.astype(np.float16),
            "pmask": pmask.astype(np.float16),
            "w1": w1,
            "brow": brow_rep,
            "pxp": swz(px, 3),
            "basep": swz(base, 3),
        })
    return in_maps


def assemble_output(results, n_total, n_tiles):
    P = 128
    valid = n_total // N_CORES
    outs = []
    for r in range(N_CORES):
        o = results[r]["out"]
        o = (o.reshape(P, n_tiles, 3).transpose(1, 0, 2)
             .reshape(n_tiles * P, 3)[:valid])
        outs.append(o)
    return np.concatenate(outs, axis=0).astype(np.float32)


_CACHED = {}


def _get_program(n_total, shard, n_tiles):
    key = (n_total, shard, n_tiles)
    if key not in _CACHED:
        _CACHED[key] = build_program(n_total, shard, n_tiles)
    return _CACHED[key]


def kernel(sampled_points, sampled_x, edge_index_filtered,
           W_concat, b_concat, W_out, b_out, W_q, b_q, W_k, b_k):
    n_total = 60000
    n_tiles = 59
    shard = n_tiles * 128
    nc = _get_program(n_total, shard, n_tiles)
    in_maps = prep_inputs(
        np.asarray(sampled_points), np.asarray(sampled_x),
        np.asarray(edge_index_filtered),
        np.asarray(W_concat), np.asarray(b_concat),
        np.asarray(W_out), np.asarray(b_out),
        np.asarray(W_q), np.asarray(b_q),
        np.asarray(W_k), np.asarray(b_k),
        n_total, shard, n_tiles)
    res = run_bass_kernel_spmd(nc, in_maps, list(range(N_CORES)))
    return assemble_output(res.results, n_total, n_tiles)
